# revision 26
# baseline (speedup 1.0000x reference)
"""Trainium2 Bass kernel for nn_Cell_First (gnn_message_passing).

Reference: 3-node NAS cell over a graph (N=50000 nodes, E=800000 edges,
D=128).  states=[h]; s_{i+1} = sum_j mixed(m_ij, states[j]);
mixed(m,x) = sum_c w[m,c]*relu(BN(branch_c(x) @ W[m,c].T + b[m,c]));
branches = (mean-neighbor-agg(x), x, h_in).  Output stack(s1,s2,s3).

Distribution (8 cores): nodes sharded by dst; edges partitioned by dst
owner.  Aggregation via dma_gather of src rows from a replicated fp8
table + one-hot TensorE matmuls accumulating agg^T in PSUM.

Key layout tricks vs the v0 kernel:
 - fp8(e3m4) gather tables packed as [P/2, 256B] node-PAIR rows; gather
   payload = 128B at 256B stride (2x cheaper descriptors), idx = table
   position >> 1 (fits int16 without hi/lo split); edges grouped by
   position parity (even/odd table views).
 - per-window shared capacities w/o 128-rounding (padding only at group
   ends); a 128-edge tile spans windows, split matmuls at window bounds.
 - state tables split in 2 chunks, AllGathered separately so next-round
   gathers overlap the collective.
 - invdeg applied at PSUM eviction (one-hot rhs holds exact 1.0 in fp8).
 - AllReduces split: state moments (fired early, hidden under AG/gather)
   vs agg moments (fired at agg end, hidden under identity/skip-branch z
   matmuls).
"""

import numpy as np
import ml_dtypes

BF16 = np.float16              # fp16 for states/weights on device
FP8 = ml_dtypes.float8_e3m4    # gather tables / one-hot rhs

N, D, E, NC = 50000, 128, 800000, 8
NLOC = N // NC                 # 6250
WIN = 32                       # dst slots per window (one-hot width)
WPB = 16                       # windows per 512-slot block
NWIN = (NLOC + WIN - 1) // WIN         # 196
NBLK = (NWIN + WPB - 1) // WPB         # 13
NT_NODES = (NLOC + 127) // 128         # 49 node-major tiles
CHL = 3072                     # local rows in table chunk 0 (tile-aligned)
CH0 = NC * CHL                 # chunk-0 table rows (24576)
CH1 = N - CH0                  # chunk-1 table rows (25424)
ZC = [512] * (NLOC // 512) + ([NLOC % 512] if NLOC % 512 else [])
EPS = 1e-5
ROUND_MS = [[0], [1, 2], [3, 4, 5]]
# moment-state ids
S_H, S_HIN, S_S1, S_S2, S_A0, S_A1, S_A2 = range(7)
GCH = 16                       # gather chunk (tiles per dma_gather call)


def _pos_of(node):
    """Table position of a node: chunk0 = per-core local rows [0,CHL),
    chunk1 = the rest, concatenated per core (AllGather layouts)."""
    k = node // NLOC
    l = node % NLOC
    return np.where(l < CHL, CHL * k + l, CH0 + (NLOC - CHL) * k + (l - CHL))


# ---------------------------------------------------------------- host prep
def preprocess(edge_index):
    """Partition/sort edges; build shared tile/run structure + per-core
    gather indices and one-hot rhs."""
    src = np.asarray(edge_index[0], dtype=np.int64)
    dst = np.asarray(edge_index[1], dtype=np.int64)
    deg = np.bincount(dst, minlength=N)
    invdeg = (1.0 / np.maximum(deg, 1.0)).astype(np.float32)

    core = dst // NLOC
    dstl = dst % NLOC
    win = dstl // WIN                     # global window 0..NWIN-1
    pos = _pos_of(src)                    # table position
    q = pos & 1                           # position parity
    ch = (src % NLOC >= CHL).astype(np.int64)   # table chunk

    # group = (win, q, ch); per-core counts -> shared caps
    gid3 = (win * 2 + q) * 2 + ch
    cgid = core * (NWIN * 4) + gid3
    cnt = np.bincount(cgid, minlength=NC * NWIN * 4).reshape(NC, NWIN, 2, 2)
    cap = cnt.max(axis=0)                 # [NWIN, 2, 2]

    # stream layout: for blk: for ch: for q: windows w of blk back-to-back.
    # Each 128-edge tile gets ONE matmul over the 32*k output cols of the
    # k consecutive windows it spans (rhs one-hot col = 32*(win-w0)+slot).
    soff = np.zeros((NWIN, 2, 2), np.int64)    # stream offset of window run
    tbase = np.zeros((NBLK, 2, 2), np.int64)   # global tile base of group
    ntq = np.zeros((NBLK, 2, 2), np.int64)     # tiles in group
    tinfo = []      # per global tile: (w0, k, rco) ; rco = rhs col offset
    w0_of = None
    t = 0
    rcols = 0
    for b in range(NBLK):
        ws = list(range(b * WPB, min((b + 1) * WPB, NWIN)))
        for c in range(2):
            for qq in range(2):
                off = 0
                wspan = {}                     # tile -> [windows]
                for w in ws:
                    soff[w, qq, c] = off
                    cw = int(cap[w, qq, c])
                    if cw == 0:
                        continue
                    for ti in range(off // 128, (off + cw - 1) // 128 + 1):
                        wspan.setdefault(ti, []).append(w)
                    off += cw
                nt = (off + 127) // 128
                tbase[b, qq, c] = t
                ntq[b, qq, c] = nt
                for i in range(nt):
                    wl = wspan.get(i, [ws[0]])
                    w0, k = wl[0], len(wl)
                    tinfo.append((w0, k, rcols))
                    rcols += WIN * k
                t += nt
    zoff = rcols          # shared all-zero 512-col rhs block (chain init)
    rcols += 512
    nt_total = t

    # per-edge placement (same formula on every core)
    gstart_key = cgid
    order = np.argsort(gstart_key, kind="stable")
    s_inv = np.empty_like(order)
    s_inv[order] = np.arange(len(order))
    counts_flat = np.bincount(gstart_key, minlength=NC * NWIN * 4)
    gstart = np.concatenate([[0], np.cumsum(counts_flat)[:-1]])
    rank = s_inv - gstart[gstart_key]     # rank within (core, win, q, ch)

    blk = win // WPB
    stream_pos = soff[win, q, ch] + rank
    tile_of = tbase[blk, q, ch] + stream_pos // 128
    part_of = stream_pos % 128
    w0_arr = np.array([ti[0] for ti in tinfo], np.int64)
    rco_arr = np.array([ti[2] for ti in tinfo], np.int64)
    col_of = rco_arr[tile_of] + WIN * (win - w0_arr[tile_of]) + dstl - win * WIN

    per_core = []
    for c0 in range(NC):
        m = core == c0
        tiles_c, parts_c = tile_of[m], part_of[m]
        idxflat = np.zeros(nt_total * 128, np.int32)
        relpos = np.where(ch[m] == 0, pos[m], pos[m] - CH0)
        idxflat[tiles_c * 128 + parts_c] = (relpos >> 1)
        assert idxflat.max() < 32768
        idxw = np.zeros((16, nt_total * 8), np.int16)
        fl = np.arange(nt_total * 128)
        idxw[fl % 16, fl // 16] = idxflat.astype(np.int16)
        idxw = np.tile(idxw, (8, 1))                     # [128, nt*8]

        rhs = np.zeros((128, rcols), np.float32)
        rhs[parts_c, col_of[m]] = 1.0
        rhs = rhs.astype(FP8)

        inv_bc = np.broadcast_to(
            invdeg[c0 * NLOC:(c0 + 1) * NLOC], (128, NLOC)).astype(BF16)
        per_core.append((idxw, rhs, np.ascontiguousarray(inv_bc)))

    return dict(tbase=tbase, ntq=ntq, tinfo=tinfo, nt=nt_total, rcols=rcols,
                zoff=zoff, per_core=per_core, invdeg=invdeg)


def make_host_inputs(h, h_in, weights, W, b, gamma, beta):
    h = np.asarray(h, np.float32)
    h_in = np.asarray(h_in, np.float32)
    # fp8 gather table for h in the chunked position layout
    perm = np.asarray(_pos_of(np.arange(N)))
    inv = np.empty(N, np.int64)
    inv[perm] = np.arange(N)
    h_tab = h[inv].astype(FP8)            # row p = h[node with pos p]
    table_h0 = h_tab[:CH0].reshape(CH0 // 2, 256)
    table_h1 = h_tab[CH0:].reshape(CH1 // 2, 256)

    wT = np.stack([W[m, c].T for m in range(6) for c in range(3)])
    wT = np.ascontiguousarray(
        wT.transpose(1, 0, 2).reshape(128, 18 * 128)).astype(BF16)
    bn = np.zeros((128, 54), np.float32)
    for m in range(6):
        for c in range(3):
            mc = m * 3 + c
            bn[:, 3 * mc + 0] = weights[m, c] * gamma[m, c]
            bn[:, 3 * mc + 1] = weights[m, c] * beta[m, c]
            bn[:, 3 * mc + 2] = b[m, c]
    per_core = []
    for k in range(NC):
        sl = slice(k * NLOC, (k + 1) * NLOC)
        per_core.append(dict(
            hT=np.ascontiguousarray(h[sl].T).astype(BF16),
            hinT=np.ascontiguousarray(h_in[sl].T).astype(BF16),
        ))
    return table_h0, table_h1, wT, bn, per_core


# ---------------------------------------------------------------- device build
def build(struct, stage=99, iters=1):
    import concourse.bass as bass
    import concourse.bacc as bacc
    import concourse.tile as tile
    import concourse.mybir as mybir

    dt = mybir.dt
    AF = mybir.ActivationFunctionType
    OP = mybir.AluOpType
    NT = struct["nt"]
    RCOLS = struct["rcols"]
    ZOFF = struct["zoff"]
    tbase, ntq, tinfo = struct["tbase"], struct["ntq"], struct["tinfo"]

    nc = bacc.Bacc("TRN2", target_bir_lowering=False, debug=False,
                   num_swdge_queues=4)

    th0_in = nc.dram_tensor("table_h0", [CH0 // 2, 256], dt.float8e3,
                            kind="ExternalInput")
    th1_in = nc.dram_tensor("table_h1", [CH1 // 2, 256], dt.float8e3,
                            kind="ExternalInput")
    idxs_in = nc.dram_tensor("idxs", [128, NT * 8], dt.int16, kind="ExternalInput")
    rhs_in = nc.dram_tensor("rhs", [128, RCOLS], dt.float8e3,
                            kind="ExternalInput")
    wT_in = nc.dram_tensor("wT", [128, 18 * 128], dt.float16, kind="ExternalInput")
    bn_in = nc.dram_tensor("bn_small", [128, 54], dt.float32, kind="ExternalInput")
    hT_in = nc.dram_tensor("hT", [128, NLOC], dt.float16, kind="ExternalInput")
    hinT_in = nc.dram_tensor("hinT", [128, NLOC], dt.float16, kind="ExternalInput")
    inv_in = nc.dram_tensor("inv_bc", [128, NLOC], dt.float16, kind="ExternalInput")
    out_cm = nc.dram_tensor("out_cm", [3, 128, NLOC], dt.float16,
                            kind="ExternalOutput")

    ARW3 = 129 * 3     # arstage blocks (h/s | hin/agg12 | agg0)
    AR_KEYS = ["hh", "a0", "s1", "a1", "s2", "a2"]
    AR_W = {"hh": 129 * 2, "a0": 129, "s1": 129, "a1": 129,
            "s2": 129, "a2": 129}

    with tile.TileContext(nc) as tc:
        import contextlib
        ctx = contextlib.ExitStack()
        with ctx:
            cst = ctx.enter_context(tc.tile_pool(name="cst", bufs=1))
            gat_p = ctx.enter_context(tc.tile_pool(name="gat", bufs=6))
            ztmp_p = ctx.enter_context(tc.tile_pool(name="ztmp", bufs=2))
            nmt_p = ctx.enter_context(tc.tile_pool(name="nmt", bufs=3))
            nm8_p = ctx.enter_context(tc.tile_pool(name="nm8", bufs=2))
            sv_p = ctx.enter_context(tc.tile_pool(name="sv", bufs=2))
            agg_ps = ctx.enter_context(tc.tile_pool(name="aggps", bufs=1, space="PSUM"))
            z_ps = ctx.enter_context(tc.tile_pool(name="zps", bufs=2, space="PSUM"))
            sm_ps = ctx.enter_context(tc.tile_pool(name="smps", bufs=1, space="PSUM"))
            dram = ctx.enter_context(tc.tile_pool(name="dram", bufs=1, space="DRAM"))

            # ---------- resident tiles ----------
            idx_sb = cst.tile([128, NT * 8], dt.int16)
            rhs_sb = cst.tile([128, RCOLS], dt.float8e3)
            wt_sb = cst.tile([128, 18 * 128], dt.float16)
            inv_sb = cst.tile([128, NLOC], dt.float16)
            xt_all = cst.tile([128, 4 * NLOC], dt.float16)
            hT = xt_all[:, 0 * NLOC:1 * NLOC]
            hinT = xt_all[:, 1 * NLOC:2 * NLOC]
            s1T = xt_all[:, 2 * NLOC:3 * NLOC]
            s2T = xt_all[:, 3 * NLOC:4 * NLOC]
            agg_all = cst.tile([128, 3 * NLOC], dt.float16)
            aggT = [agg_all[:, a * NLOC:(a + 1) * NLOC] for a in range(3)]
            acc = cst.tile([128, NLOC], dt.float16)
            # packed bf16 smalls: ident(128) c_bf(7x128) s_bf(7) ones(1)
            sb_bf = cst.tile([128, 128 + 7 * 128 + 7 + 1], dt.float16)
            ident = sb_bf[:, 0:128]
            c_bf = [sb_bf[:, 128 + 128 * s:128 + 128 * (s + 1)] for s in range(7)]
            s_bf = [sb_bf[:, 1024 + s:1025 + s] for s in range(7)]
            ones_bf = sb_bf[:, 1031:1032]
            ones8 = cst.tile([128, 1], dt.float8e3)
            # packed f32 smalls: bn(54) arstage(258) ar_sb(258) scale(18)
            # bias(18) eps(1)
            W_F32 = 54 + 2 * ARW3 + 18 + 18 + 1
            sb_f32 = cst.tile([128, W_F32], dt.float32)
            bn_sb = sb_f32[:, 0:54]
            arstage = sb_f32[:, 54:54 + ARW3]
            ar_sb = sb_f32[:, 54 + ARW3:54 + 2 * ARW3]
            bn_scale = sb_f32[:, 54 + 2 * ARW3:54 + 2 * ARW3 + 18]
            bn_bias = sb_f32[:, 54 + 2 * ARW3 + 18:54 + 2 * ARW3 + 36]
            eps_sv = sb_f32[:, 54 + 2 * ARW3 + 36:54 + 2 * ARW3 + 37]

            ag_in0 = dram.tile([CHL, D], dt.float8e3)
            ag_in1 = dram.tile([NLOC - CHL, D], dt.float8e3)
            ar_ins = {k: dram.tile([128, AR_W[k]], dt.float32,
                                   name=f"ar_in_{k}") for k in AR_KEYS}
            cur = {}

            # ---------- prep ----------
            nc.sync.dma_start(idx_sb[:], idxs_in[:])
            nc.sync.dma_start(rhs_sb[:], rhs_in[:])
            nc.sync.dma_start(wt_sb[:], wT_in[:])
            nc.sync.dma_start(bn_sb[:], bn_in[:])
            nc.sync.dma_start(hT[:], hT_in[:])
            nc.sync.dma_start(hinT[:], hinT_in[:])
            nc.sync.dma_start(inv_sb[:], inv_in[:])
            nc.gpsimd.memset(ones_bf[:], 1.0)
            nc.gpsimd.memset(ones8[:], 1.0)
            nc.gpsimd.memset(eps_sv[:], EPS)
            from concourse.masks import make_identity
            make_identity(nc, ident[:])

            def gather_raw(out_ap, in_view, c0, ct, queue=0, prepare=False,
                           sem=None):
                """dma_gather of ct tiles (128 idx each), 128B fp8 elems at
                256B stride.  Mirrors BassGpSimd.dma_gather minus the
                %256 elem-size assert (stride stays %256 as ucode needs).
                queue selects the SWDGE queue AND the Q7 core pair that
                generates descriptors (ucode: cpu_id/2 == queue_num), so
                gathers on different queues descgen in parallel.  With
                prepare=True emits a PREPARE_ONLY prep (descgen now, DMA
                fires at the matching trigger_dma on the same queue)."""
                g = nc.gpsimd
                n_idx = ct * 128
                idxs_ap = idx_sb[:, 8 * c0:8 * (c0 + ct)]
                _in_ap = g.lower_ap_dma(in_view, for_custom_bir_dma=True)
                inst = g.add_instruction(mybir.InstDMAGatherAnt(
                    name=g.bass.get_next_instruction_name(),
                    ins=[*_in_ap, g.lower_ap(idxs_ap),
                         g.lower_val_access(g.to_reg(n_idx))],
                    outs=[g.lower_ap(out_ap)],
                    transpose=False, num_idxs=n_idx, elem_size=128,
                    stride_bytes_256=1, gen_mode=int(prepare),
                    single_packet=False,
                    queue_num=queue, sbuf_tokens_per_rank=0,
                    sbuf_free_dim_per_rank=0, sbuf_free_dim_pad_per_rank=0,
                    sbuf_byte_offset=0))
                if prepare:
                    inst.then_inc(sem, 16)
                    return g._track_prepare_only(inst, queue)
                return inst

            def moment_psums():
                mta = sm_ps.tile([128, 129], dt.float32, space="PSUM",
                                 tag="momCSa", bufs=1, name="mta")
                mtb = sm_ps.tile([128, 129], dt.float32, space="PSUM",
                                 tag="momCSb", bufs=1, name="mtb")
                return (mta, mtb)

            def moment_acc(mts, xt, t):
                # alternate between two psum accumulate chains (dependency
                # stalls on same-psum matmul chains cost ~300ns/inst on HW)
                mt = mts[t & 1]
                cps, sps = mt[:, 0:128], mt[:, 128:129]
                last = t >= NT_NODES - 2
                nc.tensor.matmul(cps, lhsT=xt, rhs=xt,
                                 start=(t < 2), stop=last,
                                 skip_group_check=True)
                nc.tensor.matmul(sps, lhsT=xt, rhs=ones8[:],
                                 start=False, stop=last,
                                 skip_group_check=True)

            def moment_evict(mts, stg, blk):
                co = 129 * blk
                mrgb = ztmp_p.tile([128, 129], dt.float32, tag="mrgb")
                nc.scalar.activation(mrgb[:], mts[1][:], AF.Copy, bias=0.0,
                                     scale=1.0)
                mrg = ztmp_p.tile([128, 129], dt.float32, tag="mrg")
                nc.vector.tensor_tensor(out=mrg[:], in0=mts[0][:],
                                        in1=mrgb[:], op=OP.add)
                nc.vector.tensor_scalar_mul(stg[:, co:co + 129], mrg[:],
                                            1.0 / N)

            def moments_from_cm(xT, stg, blk, sink=None):
                """Moments of a ch-major [128, NLOC] state via PE transposes.
                sink(tn, nm_ap, wv) optionally consumes node-major tiles."""
                mts = moment_psums()
                pend = []

                def _consume(tn, tp, wv):
                    nm8 = nm8_p.tile([128, 128], dt.float8e3, tag="nm8")
                    if wv < 128:
                        nc.vector.memset(nm8[96:, :], 0)
                    nc.scalar.activation(nm8[:wv, :], tp[:wv, :], AF.Copy,
                                         bias=0.0, scale=1.0)
                    moment_acc(mts, nm8[:], tn)
                    if sink is not None:
                        sink(tn, nm8, wv)

                for tn in range(NT_NODES):
                    wv = min(128, NLOC - 128 * tn)
                    tp = sm_ps.tile([128, 128], dt.float16, space="PSUM", tag="tp",
                                    bufs=2)
                    nc.tensor.transpose(tp[:wv, :], xT[:, 128 * tn:128 * tn + wv],
                                        ident[:])
                    pend.append((tn, tp, wv))
                    if len(pend) > 1:
                        _consume(*pend.pop(0))
                _consume(*pend.pop(0))
                moment_evict(mts, stg, blk)

            # ---------- gather prefetch machinery ----------
            # Shared per-pass call plan: same structure every pass, greedy
            # per-queue tile balance.  Preps (descgen on the queue's Q7
            # pair) are emitted ahead of time via pump(); the agg phase
            # fires them with trigger_dma in the same per-queue order.
            call_plan = []        # (c, qpar, b, t0g0, ct, qsel)
            _gq = [0, 0, 0, 0]
            for _c in range(2):
                for _b in range(NBLK):
                    for _qp in range(2):
                        _t0, _ntg = int(tbase[_b, _qp, _c]), int(ntq[_b, _qp, _c])
                        if _ntg == 0:
                            continue
                        _ncall = (_ntg + GCH - 1) // GCH
                        _g0 = 0
                        for _i in range(_ncall):
                            _ct = _ntg // _ncall + (1 if _i < _ntg % _ncall
                                                    else 0)
                            _qs = min(range(4), key=lambda j: _gq[j])
                            _gq[_qs] += _ct
                            call_plan.append((_c, _qp, _b, _t0 + _g0, _ct, _qs))
                            _g0 += _ct
            def queue_phase(tbl0, tbl1):
                """Record the table views for the next agg pass."""
                views = [(tbl0[:, 0:128], tbl0[:, 128:256]),
                         (tbl1[:, 0:128], tbl1[:, 128:256])]
                return views

            # ---------- aggregation ----------
            def agg_phase(a, views, sidework=None):
                """agg into aggT[a]: per-call dma_gather (round-robin SWDGE
                queues, descgen parallel on 4 Q7 pairs) + one-hot matmuls
                accumulating in PSUM.  Pass 0 evicts invdeg-scaled partials
                into aggT (fp16); pass 1 adds on top."""
                dst = aggT[a]
                mts = moment_psums() if stage != 2 else None
                mom_state = [0]

                def mom_flush(upto):
                    while mom_state[0] * 128 < upto:
                        tn = mom_state[0]
                        wv = min(128, NLOC - 128 * tn)
                        tp = sm_ps.tile([128, 128], dt.float16, space="PSUM",
                                        tag="tp", bufs=2)
                        nc.tensor.transpose(tp[:wv, :],
                                            dst[:, 128 * tn:128 * tn + wv],
                                            ident[:])
                        nm8 = nm8_p.tile([128, 128], dt.float8e3, tag="nm8")
                        if wv < 128:
                            nc.vector.memset(nm8[96:, :], 0)
                        nc.scalar.activation(nm8[:wv, :], tp[:wv, :], AF.Copy,
                                             bias=0.0, scale=1.0)
                        moment_acc(mts, nm8[:], tn)
                        mom_state[0] += 1

                ci = 0
                for c in range(2):
                    for b in range(NBLK):
                        Pa = agg_ps.tile([128, 512], dt.float32, space="PSUM",
                                         tag="Pa", name="Pa")
                        Pb = agg_ps.tile([128, 512], dt.float32, space="PSUM",
                                         tag="Pb", name="Pb")
                        Ps = (Pa, Pb)
                        nmm = int(ntq[b, 0, c] + ntq[b, 1, c])
                        wsn = min((b + 1) * WPB, NWIN) - b * WPB
                        for Px in (Pa, Pb):
                            nc.tensor.matmul(
                                Px[:, 0:WIN * wsn], lhsT=rhs_sb[:, 0:128],
                                rhs=rhs_sb[:, ZOFF:ZOFF + WIN * wsn],
                                start=True, stop=False, skip_group_check=True)
                        done = 0
                        while ci < len(call_plan) and call_plan[ci][0] == c \
                                and call_plan[ci][2] == b:
                            _, qp, _, t0, ct, qs = call_plan[ci]
                            g = gat_p.tile([128, ct, 128], dt.float8e3,
                                           tag="gat")
                            gather_raw(g[:], views[c][qp], t0, ct, queue=qs)
                            for i in range(ct):
                                t = t0 + i
                                w0, k, rco = tinfo[t]
                                co = WIN * (w0 - b * WPB)
                                nc.tensor.matmul(
                                    Ps[done & 1][:, co:co + WIN * k],
                                    lhsT=g[:, i, :],
                                    rhs=rhs_sb[:, rco:rco + WIN * k],
                                    start=False,
                                    stop=(done >= nmm - 2),
                                    skip_group_check=True)
                                done += 1
                            ci += 1
                        nbc = min(512, NLOC - 512 * b)
                        dchunk = dst[:, 512 * b:512 * b + nbc]
                        cpb = ztmp_p.tile([128, 512], dt.float16, tag="psum2",
                                          name="cpb")
                        nc.scalar.activation(cpb[:, :nbc], Pb[:, :nbc], AF.Copy,
                                             bias=0.0, scale=1.0)
                        ps_sum = ztmp_p.tile([128, 512], dt.float16, tag="psum3",
                                             name="ps_sum")
                        nc.vector.tensor_tensor(
                            out=ps_sum[:, :nbc], in0=Pa[:, :nbc],
                            in1=cpb[:, :nbc], op=OP.add)
                        if c == 0:
                            nc.vector.tensor_tensor(
                                out=dchunk, in0=ps_sum[:, :nbc],
                                in1=inv_sb[:, 512 * b:512 * b + nbc], op=OP.mult)
                            continue
                        pt = ztmp_p.tile([128, 512], dt.float16, tag="zt")
                        nc.vector.tensor_tensor(
                            out=pt[:, :nbc], in0=ps_sum[:, :nbc],
                            in1=inv_sb[:, 512 * b:512 * b + nbc], op=OP.mult)
                        nc.vector.tensor_tensor(
                            out=dchunk, in0=dchunk, in1=pt[:, :nbc], op=OP.add)
                        if stage != 2 and b > 0:
                            mom_flush(512 * b)     # lag one block
                        if sidework:
                            sidework.pop(0)()
                if stage != 2:
                    mom_flush(NLOC)
                return mts

            # ---------- collectives ----------
            def fire_ar(ar_in, ar_out, stg_ap):
                nc.sync.dma_start(ar_in[:], stg_ap)
                nc.gpsimd.collective_compute(
                    "AllReduce", OP.add, replica_groups=[list(range(NC))],
                    ins=[ar_in.opt()], outs=[ar_out.opt()])

            def unpack_ar(ar_out, blocks):
                """blocks: list of (block idx in ar_out, state id)"""
                w = 129 * len(blocks)
                nc.sync.dma_start(ar_sb[:, :w], ar_out[:])
                for i, (blk, sid) in enumerate(blocks):
                    co = 129 * i
                    nc.vector.tensor_copy(c_bf[sid][:], ar_sb[:, co:co + 128])
                    nc.vector.tensor_copy(s_bf[sid][:], ar_sb[:, co + 128:co + 129])

            # ---------- BN params for one (m, c) ----------
            def bn_params(mc, sid):
                wt = wt_sb[:, 128 * mc:128 * (mc + 1)]
                wgam = bn_sb[:, 3 * mc + 0:3 * mc + 1]
                wbeta = bn_sb[:, 3 * mc + 1:3 * mc + 2]
                blin = bn_sb[:, 3 * mc + 2:3 * mc + 3]
                vps = sm_ps.tile([128, 128], dt.float32, space="PSUM",
                                 tag="tp", bufs=2)
                nc.tensor.matmul(vps[:], lhsT=c_bf[sid][:], rhs=wt, start=True,
                                 stop=True)
                vsb = nmt_p.tile([128, 128], dt.float16, tag="nmt")
                nc.vector.tensor_copy(vsb[:], vps[:])
                msb = nmt_p.tile([128, 128], dt.float16, tag="nmt")
                nc.vector.tensor_tensor(out=msb[:], in0=vsb[:], in1=wt, op=OP.mult)
                bnv = sm_ps.tile([128, 2], dt.float32, space="PSUM",
                                 tag="tp", bufs=2, name="bnv")
                d2, ws_ = bnv[:, 0:1], bnv[:, 1:2]
                nc.tensor.matmul(d2, lhsT=msb[:], rhs=ones_bf[:], start=True,
                                 stop=True, skip_group_check=True)
                nc.tensor.matmul(ws_, lhsT=wt, rhs=s_bf[sid][:], start=False,
                                 stop=True, skip_group_check=True)
                sv = sv_p.tile([128, 10], dt.float32, tag="sv")
                mu, t1, t2, e2, mu2, var, sd, rv, t4, t5 = (
                    sv[:, i:i + 1] for i in range(10))
                nc.vector.tensor_tensor(out=mu, in0=ws_[:], in1=blin, op=OP.add)
                nc.vector.tensor_scalar(out=t1, in0=ws_[:], scalar1=2.0,
                                        scalar2=blin, op0=OP.mult, op1=OP.add)
                nc.vector.tensor_tensor(out=t2, in0=t1, in1=blin, op=OP.mult)
                nc.vector.tensor_tensor(out=e2, in0=d2[:], in1=t2, op=OP.add)
                nc.vector.tensor_tensor(out=mu2, in0=mu, in1=mu, op=OP.mult)
                nc.vector.tensor_tensor(out=var, in0=e2, in1=mu2, op=OP.subtract)
                nc.scalar.activation(sd, var, AF.Sqrt, bias=eps_sv, scale=1.0)
                nc.vector.reciprocal(rv, sd)
                nc.vector.tensor_tensor(out=bn_scale[:, mc:mc + 1], in0=wgam,
                                        in1=rv, op=OP.mult)
                nc.vector.tensor_tensor(out=t4, in0=blin, in1=mu, op=OP.subtract)
                nc.vector.tensor_tensor(out=t5, in0=bn_scale[:, mc:mc + 1],
                                        in1=t4, op=OP.mult)
                nc.vector.tensor_tensor(out=bn_bias[:, mc:mc + 1], in0=wbeta,
                                        in1=t5, op=OP.add)

            # ---------- z partials ----------
            zstate = {"first": [True] * len(ZC)}

            def z_chunk(r, m, c, ci):
                j = m - ROUND_MS[r][0]
                mc = m * 3 + c
                rx = (aggT[j] if c == 0 else
                      [hT, s1T, s2T][j] if c == 1 else hinT)
                lw = wt_sb[:, 128 * mc:128 * (mc + 1)]
                off, cw = 512 * ci, ZC[ci]
                zp = z_ps.tile([128, 512], dt.float32, space="PSUM",
                               tag="zp")
                nc.tensor.matmul(zp[:, :cw], lhsT=lw,
                                 rhs=rx[:, off:off + cw],
                                 start=True, stop=True)
                if zstate["first"][ci]:
                    nc.scalar.activation(
                        acc[:, off:off + cw], zp[:, :cw], AF.Relu,
                        bias=bn_bias[:, mc:mc + 1],
                        scale=bn_scale[:, mc:mc + 1])
                    zstate["first"][ci] = False
                else:
                    zt = ztmp_p.tile([128, 512], dt.float16, tag="zt")
                    nc.scalar.activation(
                        zt[:, :cw], zp[:, :cw], AF.Relu,
                        bias=bn_bias[:, mc:mc + 1],
                        scale=bn_scale[:, mc:mc + 1])
                    nc.vector.tensor_tensor(
                        out=acc[:, off:off + cw],
                        in0=acc[:, off:off + cw],
                        in1=zt[:, :cw], op=OP.add)

            def z_partial(r, pairs):
                for m, c in pairs:
                    for ci in range(len(ZC)):
                        z_chunk(r, m, c, ci)

            # ---------- fused round tail: z c0 + out + incremental
            # finalize (sT copy, moments, fp8 cast, chunked AllGather) ----
            def finish_round(r):
                """c0 z-matmuls chunk by chunk; finalize work (sT copy,
                transpose, moments, fp8 rows) LAGS one chunk so engines
                pipeline instead of ping-ponging; AG0 fires once the first
                CHL rows are on DRAM (r<2)."""
                ms = ROUND_MS[r]
                sT = [s1T, s2T][r] if r < 2 else None
                mts = moment_psums() if r < 2 else None
                fin_state = [0]

                def fin_flush(upto):
                    while fin_state[0] * 128 < upto:
                        tn = fin_state[0]
                        wv = min(128, NLOC - 128 * tn)
                        tp = sm_ps.tile([128, 128], dt.float16,
                                        space="PSUM", tag="tp", bufs=2)
                        nc.tensor.transpose(
                            tp[:wv, :], sT[:, 128 * tn:128 * tn + wv],
                            ident[:])
                        nm8 = nm8_p.tile([128, 128], dt.float8e3, tag="nm8")
                        if wv < 128:
                            nc.vector.memset(nm8[96:, :], 0)
                        nc.vector.tensor_copy(nm8[:wv, :], tp[:wv, :])
                        moment_acc(mts, nm8[:], tn)
                        n0 = 128 * tn
                        if n0 + wv <= CHL:
                            nc.sync.dma_start(ag_in0[n0:n0 + wv, :],
                                              nm8[:wv, :])
                        else:
                            nc.sync.dma_start(
                                ag_in1[n0 - CHL:n0 - CHL + wv, :],
                                nm8[:wv, :])
                        fin_state[0] += 1
                        if fin_state[0] * 128 == CHL:
                            t0, _ = cur["tbls"][r]
                            nc.gpsimd.collective_compute(
                                "AllGather", OP.bypass,
                                replica_groups=[list(range(NC))],
                                ins=[ag_in0.opt()], outs=[t0.opt()])

                off = 0
                for ci, cw in enumerate(ZC):
                    for j, m in enumerate(ms):
                        mc = m * 3 + 0
                        lw = wt_sb[:, 128 * mc:128 * (mc + 1)]
                        zp = z_ps.tile([128, 512], dt.float32, space="PSUM",
                                       tag="zp")
                        nc.tensor.matmul(zp[:, :cw], lhsT=lw,
                                         rhs=aggT[j][:, off:off + cw],
                                         start=True, stop=True)
                        zt = ztmp_p.tile([128, 512], dt.float16, tag="zt")
                        nc.scalar.activation(zt[:, :cw], zp[:, :cw], AF.Relu,
                                             bias=bn_bias[:, mc:mc + 1],
                                             scale=bn_scale[:, mc:mc + 1])
                        nc.vector.tensor_tensor(
                            out=acc[:, off:off + cw], in0=acc[:, off:off + cw],
                            in1=zt[:, :cw], op=OP.add)
                    nc.sync.dma_start(out_cm[r][:, off:off + cw],
                                      acc[:, off:off + cw])
                    if r < 2:
                        nc.vector.tensor_copy(sT[:, off:off + cw],
                                              acc[:, off:off + cw])
                        if ci > 0:
                            fin_flush(off)        # lag one chunk
                    off += cw
                if r < 2:
                    fin_flush(NLOC)
                    _, t1_ = cur["tbls"][r]
                    nc.gpsimd.collective_compute(
                        "AllGather", OP.bypass, replica_groups=[list(range(NC))],
                        ins=[ag_in1.opt()], outs=[t1_.opt()])
                    moment_evict(mts, arstage, 0)
                    key = f"s{r + 1}"
                    fire_ar(ar_ins[key], cur["ar"][key], arstage[:, 0:129])
                    return cur["tbls"][r]

            def agg_and_bn(r, views, sidework=None):
                """agg phase r (with interleaved sidework on the otherwise
                descgen-bound stretch), then the agg-moment AR; leftover
                sidework flushes after the AR fires (its latency shadow)."""
                mts = agg_phase(r, views, sidework)
                blk = 2 if r == 0 else 1
                key = f"a{r}"
                if stage != 2:
                    moment_evict(mts, arstage, blk)
                if stage not in (0, 2):
                    fire_ar(ar_ins[key], cur["ar"][key],
                            arstage[:, 129 * blk:129 * (blk + 1)])
                while sidework:
                    sidework.pop(0)()

            # ================= main schedule =================
            def emit(it, last):
                zstate["first"] = [True] * len(ZC)
                s0 = queue_phase(th0_in[:, :], th1_in[:, :])
                moments_from_cm(hT, arstage, 0)
                moments_from_cm(hinT, arstage, 1)
                if stage not in (0, 2):
                    fire_ar(ar_ins["hh"], cur["ar"]["hh"], arstage[:, 0:258])

                # round 0: identity/skip z-branches run as sidework inside
                # the (descgen-bound) agg phase, once the early h/hin
                # moment-AR lands.
                s1 = queue_phase(*cur["tbls"][0])
                sw = []
                if stage not in (0, 1, 2):
                    def _unp0():
                        unpack_ar(cur["ar"]["hh"], [(0, S_H), (1, S_HIN)])
                        bn_params(1, S_H)
                        bn_params(2, S_HIN)
                    sw.append(lambda: None)   # let AR-hh land first
                    sw.append(_unp0)
                    for ci in range(len(ZC)):
                        sw.append(lambda ci=ci: (z_chunk(0, 0, 1, ci),
                                                 z_chunk(0, 0, 2, ci)))
                agg_and_bn(0, s0, sw)
                if stage in (0, 1, 2):
                    off = 0
                    for cw in ZC:
                        nc.vector.tensor_copy(acc[:, off:off + cw],
                                              aggT[0][:, off:off + cw])
                        off += cw
                    nc.sync.dma_start(out_cm[0], acc[:])
                    return
                unpack_ar(cur["ar"]["a0"], [(0, S_A0)])
                bn_params(0, S_A0)
                tbl10, tbl11 = finish_round(0)
                if stage == 3:
                    return
                zstate["first"] = [True] * len(ZC)

                # round 1: identity branch on h under AG+gather; the rest as
                # agg sidework (s1-stat unpack a few blocks in, once AR-s1
                # lands).
                bn_params(1 * 3 + 1, S_H)
                bn_params(1 * 3 + 2, S_HIN)
                bn_params(2 * 3 + 2, S_HIN)
                z_partial(1, [(1, 1)])
                s2 = queue_phase(*cur["tbls"][1])
                sw = [lambda ci=ci: z_chunk(1, 1, 2, ci)
                      for ci in range(len(ZC))]

                def _unp1():
                    unpack_ar(cur["ar"]["s1"], [(0, S_S1)])
                    bn_params(2 * 3 + 1, S_S1)
                sw.insert(4, _unp1)
                sw += [lambda ci=ci: z_chunk(1, 2, 2, ci)
                       for ci in range(len(ZC))]
                sw += [lambda ci=ci: z_chunk(1, 2, 1, ci)
                       for ci in range(len(ZC))]
                agg_and_bn(1, s1, sw)
                unpack_ar(cur["ar"]["a1"], [(0, S_A1)])
                bn_params(1 * 3 + 0, S_A0)
                bn_params(2 * 3 + 0, S_A1)
                tbl20, tbl21 = finish_round(1)
                zstate["first"] = [True] * len(ZC)

                # round 2
                bn_params(3 * 3 + 1, S_H)
                bn_params(3 * 3 + 2, S_HIN)
                bn_params(4 * 3 + 1, S_S1)
                bn_params(4 * 3 + 2, S_HIN)
                bn_params(5 * 3 + 2, S_HIN)
                z_partial(2, [(3, 1), (4, 1)])
                sw = [lambda ci=ci: z_chunk(2, 3, 2, ci)
                      for ci in range(len(ZC))]

                def _unp2():
                    unpack_ar(cur["ar"]["s2"], [(0, S_S2)])
                    bn_params(5 * 3 + 1, S_S2)
                sw.insert(4, _unp2)
                sw += [lambda ci=ci: z_chunk(2, 4, 2, ci)
                       for ci in range(len(ZC))]
                sw += [lambda ci=ci: z_chunk(2, 5, 2, ci)
                       for ci in range(len(ZC))]
                sw += [lambda ci=ci: z_chunk(2, 5, 1, ci)
                       for ci in range(len(ZC))]
                agg_and_bn(2, s2, sw)
                unpack_ar(cur["ar"]["a2"], [(0, S_A2)])
                bn_params(3 * 3 + 0, S_A0)
                bn_params(4 * 3 + 0, S_A1)
                bn_params(5 * 3 + 0, S_A2)
                finish_round(2)

            for it in range(iters):
                cur["tbls"] = [
                    (dram.tile([CH0 // 2, 256], dt.float8e3, addr_space="Shared",
                               name=f"tbl{r}0_{it}"),
                     dram.tile([CH1 // 2, 256], dt.float8e3, addr_space="Shared",
                               name=f"tbl{r}1_{it}"))
                    for r in range(2)]
                cur["ar"] = {k: dram.tile([128, AR_W[k]], dt.float32,
                                          addr_space="Shared",
                                          name=f"ar_{k}_{it}")
                             for k in AR_KEYS}
                emit(it, it == iters - 1)

    nc.compile()
    return nc


# ---------------------------------------------------------------- entry point
def prepare(edge_index, h, h_in, weights, W, b, gamma, beta, stage=99, iters=1):
    struct = preprocess(edge_index)
    th0, th1, wT, bn, per_core_s = make_host_inputs(
        h, h_in, weights, W, b, gamma, beta)
    in_maps = []
    for k in range(NC):
        idxw, rhs, inv_bc = struct["per_core"][k]
        in_maps.append(dict(
            table_h0=th0, table_h1=th1, idxs=idxw, rhs=rhs, wT=wT,
            bn_small=bn, inv_bc=inv_bc,
            hT=per_core_s[k]["hT"], hinT=per_core_s[k]["hinT"]))
    nc = build(struct, stage=stage, iters=iters)
    return nc, in_maps


def assemble(results):
    out = np.empty((3, N, D), np.float32)
    for k in range(NC):
        cm = results[k]["out_cm"]
        for r in range(3):
            out[r, k * NLOC:(k + 1) * NLOC, :] = cm[r].T
    return out


def kernel(edge_index, h, h_in, weights, W, b, gamma, beta):
    from concourse.bass_utils import run_bass_kernel_spmd
    nc, in_maps = prepare(np.asarray(edge_index), h, h_in,
                          np.asarray(weights, np.float32),
                          np.asarray(W, np.float32), np.asarray(b, np.float32),
                          np.asarray(gamma, np.float32),
                          np.asarray(beta, np.float32))
    res = run_bass_kernel_spmd(nc, in_maps, core_ids=list(range(NC)))
    return assemble(res.results)



# revision 30
# speedup vs baseline: 1.2068x; 1.2068x over previous
"""Trainium2 Bass kernel for nn_Cell_First (gnn_message_passing).

Reference: 3-node NAS cell over a graph (N=50000 nodes, E=800000 edges,
D=128).  states=[h]; s_{i+1} = sum_j mixed(m_ij, states[j]);
mixed(m,x) = sum_c w[m,c]*relu(BN(branch_c(x) @ W[m,c].T + b[m,c]));
branches = (mean-neighbor-agg(x), x, h_in).  Output stack(s1,s2,s3).

Distribution (8 cores): nodes sharded by dst; edges partitioned by dst
owner.  Aggregation via dma_gather of src rows from a replicated fp8
table + one-hot TensorE matmuls accumulating agg^T in PSUM.

Key layout tricks vs the v0 kernel:
 - fp8(e3m4) gather tables packed as [P/2, 256B] node-PAIR rows; gather
   payload = 128B at 256B stride (2x cheaper descriptors), idx = table
   position >> 1 (fits int16 without hi/lo split); edges grouped by
   position parity (even/odd table views).
 - per-window shared capacities w/o 128-rounding (padding only at group
   ends); a 128-edge tile spans windows, split matmuls at window bounds.
 - state tables split in 2 chunks, AllGathered separately so next-round
   gathers overlap the collective.
 - invdeg applied at PSUM eviction (one-hot rhs holds exact 1.0 in fp8).
 - AllReduces split: state moments (fired early, hidden under AG/gather)
   vs agg moments (fired at agg end, hidden under identity/skip-branch z
   matmuls).
"""

import numpy as np
import ml_dtypes

BF16 = np.float16              # fp16 for states/weights on device
FP8 = ml_dtypes.float8_e3m4    # gather tables / one-hot rhs

N, D, E, NC = 50000, 128, 800000, 8
NLOC = N // NC                 # 6250
WIN = 32                       # dst slots per window (one-hot width)
WPB = 16                       # windows per 512-slot block
NWIN = (NLOC + WIN - 1) // WIN         # 196
NBLK = (NWIN + WPB - 1) // WPB         # 13
NT_NODES = (NLOC + 127) // 128         # 49 node-major tiles
CHL = 3072                     # local rows in table chunk 0 (tile-aligned)
CH0 = NC * CHL                 # chunk-0 table rows (24576)
CH1 = N - CH0                  # chunk-1 table rows (25424)
ZC = [512] * (NLOC // 512) + ([NLOC % 512] if NLOC % 512 else [])
EPS = 1e-5
ROUND_MS = [[0], [1, 2], [3, 4, 5]]
# moment-state ids
S_H, S_HIN, S_S1, S_S2, S_A0, S_A1, S_A2 = range(7)
GCH = 16                       # gather chunk (tiles per dma_gather call)


def _pos_of(node):
    """Table position of a node: chunk0 = per-core local rows [0,CHL),
    chunk1 = the rest, concatenated per core (AllGather layouts)."""
    k = node // NLOC
    l = node % NLOC
    return np.where(l < CHL, CHL * k + l, CH0 + (NLOC - CHL) * k + (l - CHL))


# ---------------------------------------------------------------- host prep
def preprocess(edge_index):
    """Partition/sort edges; build shared tile/run structure + per-core
    gather indices and one-hot rhs."""
    src = np.asarray(edge_index[0], dtype=np.int64)
    dst = np.asarray(edge_index[1], dtype=np.int64)
    deg = np.bincount(dst, minlength=N)
    invdeg = (1.0 / np.maximum(deg, 1.0)).astype(np.float32)

    core = dst // NLOC
    dstl = dst % NLOC
    win = dstl // WIN                     # global window 0..NWIN-1
    pos = _pos_of(src)                    # table position
    q = pos & 1                           # position parity
    ch = (src % NLOC >= CHL).astype(np.int64)   # table chunk

    # group = (win, q, ch); per-core counts -> shared caps
    gid3 = (win * 2 + q) * 2 + ch
    cgid = core * (NWIN * 4) + gid3
    cnt = np.bincount(cgid, minlength=NC * NWIN * 4).reshape(NC, NWIN, 2, 2)
    cap = cnt.max(axis=0)                 # [NWIN, 2, 2]

    # stream layout: for blk: for ch: for q: windows w of blk back-to-back.
    # Each 128-edge tile gets ONE matmul over the 32*k output cols of the
    # k consecutive windows it spans (rhs one-hot col = 32*(win-w0)+slot).
    soff = np.zeros((NWIN, 2, 2), np.int64)    # stream offset of window run
    tbase = np.zeros((NBLK, 2, 2), np.int64)   # global tile base of group
    ntq = np.zeros((NBLK, 2, 2), np.int64)     # tiles in group
    tinfo = []      # per global tile: (w0, k, rco) ; rco = rhs col offset
    w0_of = None
    t = 0
    rcols = 0
    for b in range(NBLK):
        ws = list(range(b * WPB, min((b + 1) * WPB, NWIN)))
        for c in range(2):
            for qq in range(2):
                off = 0
                wspan = {}                     # tile -> [windows]
                for w in ws:
                    soff[w, qq, c] = off
                    cw = int(cap[w, qq, c])
                    if cw == 0:
                        continue
                    for ti in range(off // 128, (off + cw - 1) // 128 + 1):
                        wspan.setdefault(ti, []).append(w)
                    off += cw
                nt = (off + 127) // 128
                tbase[b, qq, c] = t
                ntq[b, qq, c] = nt
                for i in range(nt):
                    wl = wspan.get(i, [ws[0]])
                    w0, k = wl[0], len(wl)
                    tinfo.append((w0, k, rcols))
                    rcols += WIN * k
                t += nt
    zoff = rcols          # shared all-zero 512-col rhs block (chain init)
    rcols += 512
    nt_total = t

    # per-edge placement (same formula on every core)
    gstart_key = cgid
    order = np.argsort(gstart_key, kind="stable")
    s_inv = np.empty_like(order)
    s_inv[order] = np.arange(len(order))
    counts_flat = np.bincount(gstart_key, minlength=NC * NWIN * 4)
    gstart = np.concatenate([[0], np.cumsum(counts_flat)[:-1]])
    rank = s_inv - gstart[gstart_key]     # rank within (core, win, q, ch)

    blk = win // WPB
    stream_pos = soff[win, q, ch] + rank
    tile_of = tbase[blk, q, ch] + stream_pos // 128
    part_of = stream_pos % 128
    w0_arr = np.array([ti[0] for ti in tinfo], np.int64)
    rco_arr = np.array([ti[2] for ti in tinfo], np.int64)
    col_of = rco_arr[tile_of] + WIN * (win - w0_arr[tile_of]) + dstl - win * WIN

    per_core = []
    for c0 in range(NC):
        m = core == c0
        tiles_c, parts_c = tile_of[m], part_of[m]
        idxflat = np.zeros(nt_total * 128, np.int32)
        relpos = np.where(ch[m] == 0, pos[m], pos[m] - CH0)
        idxflat[tiles_c * 128 + parts_c] = (relpos >> 1)
        assert idxflat.max() < 32768
        idxw = np.zeros((16, nt_total * 8), np.int16)
        fl = np.arange(nt_total * 128)
        idxw[fl % 16, fl // 16] = idxflat.astype(np.int16)
        idxw = np.tile(idxw, (8, 1))                     # [128, nt*8]

        rhs = np.zeros((128, rcols), np.float32)
        rhs[parts_c, col_of[m]] = 1.0
        rhs = rhs.astype(FP8)

        inv_bc = np.broadcast_to(
            invdeg[c0 * NLOC:(c0 + 1) * NLOC], (128, NLOC)).astype(BF16)
        per_core.append((idxw, rhs, np.ascontiguousarray(inv_bc)))

    return dict(tbase=tbase, ntq=ntq, tinfo=tinfo, nt=nt_total, rcols=rcols,
                zoff=zoff, per_core=per_core, invdeg=invdeg)


def make_host_inputs(h, h_in, weights, W, b, gamma, beta):
    h = np.asarray(h, np.float32)
    h_in = np.asarray(h_in, np.float32)
    # fp8 gather table for h in the chunked position layout
    perm = np.asarray(_pos_of(np.arange(N)))
    inv = np.empty(N, np.int64)
    inv[perm] = np.arange(N)
    h_tab = h[inv].astype(FP8)            # row p = h[node with pos p]
    table_h0 = h_tab[:CH0].reshape(CH0 // 2, 256)
    table_h1 = h_tab[CH0:].reshape(CH1 // 2, 256)

    wT = np.stack([W[m, c].T for m in range(6) for c in range(3)])
    wT = np.ascontiguousarray(
        wT.transpose(1, 0, 2).reshape(128, 18 * 128)).astype(BF16)
    bn = np.zeros((128, 54), np.float32)
    for m in range(6):
        for c in range(3):
            mc = m * 3 + c
            bn[:, 3 * mc + 0] = weights[m, c] * gamma[m, c]
            bn[:, 3 * mc + 1] = weights[m, c] * beta[m, c]
            bn[:, 3 * mc + 2] = b[m, c]
    per_core = []
    for k in range(NC):
        sl = slice(k * NLOC, (k + 1) * NLOC)
        per_core.append(dict(
            hT=np.ascontiguousarray(h[sl].T).astype(BF16),
            hinT=np.ascontiguousarray(h_in[sl].T).astype(BF16),
        ))
    return table_h0, table_h1, wT, bn, per_core


# ---------------------------------------------------------------- device build
def build(struct, stage=99, iters=1):
    import concourse.bass as bass
    import concourse.bacc as bacc
    import concourse.tile as tile
    import concourse.mybir as mybir

    dt = mybir.dt
    AF = mybir.ActivationFunctionType
    OP = mybir.AluOpType
    NT = struct["nt"]
    RCOLS = struct["rcols"]
    ZOFF = struct["zoff"]
    tbase, ntq, tinfo = struct["tbase"], struct["ntq"], struct["tinfo"]

    nc = bacc.Bacc("TRN2", target_bir_lowering=False, debug=False,
                   num_swdge_queues=4)

    th0_in = nc.dram_tensor("table_h0", [CH0 // 2, 256], dt.float8e3,
                            kind="ExternalInput")
    th1_in = nc.dram_tensor("table_h1", [CH1 // 2, 256], dt.float8e3,
                            kind="ExternalInput")
    idxs_in = nc.dram_tensor("idxs", [128, NT * 8], dt.int16, kind="ExternalInput")
    rhs_in = nc.dram_tensor("rhs", [128, RCOLS], dt.float8e3,
                            kind="ExternalInput")
    wT_in = nc.dram_tensor("wT", [128, 18 * 128], dt.float16, kind="ExternalInput")
    bn_in = nc.dram_tensor("bn_small", [128, 54], dt.float32, kind="ExternalInput")
    hT_in = nc.dram_tensor("hT", [128, NLOC], dt.float16, kind="ExternalInput")
    hinT_in = nc.dram_tensor("hinT", [128, NLOC], dt.float16, kind="ExternalInput")
    inv_in = nc.dram_tensor("inv_bc", [128, NLOC], dt.float16, kind="ExternalInput")
    out_cm = nc.dram_tensor("out_cm", [3, 128, NLOC], dt.float16,
                            kind="ExternalOutput")

    ARW3 = 129 * 3     # arstage blocks (h/s | hin/agg12 | agg0)
    AR_KEYS = ["hh", "a0", "s1", "a1", "s2", "a2"]
    AR_W = {"hh": 129 * 2, "a0": 129, "s1": 129, "a1": 129,
            "s2": 129, "a2": 129}

    with tile.TileContext(nc) as tc:
        import contextlib
        ctx = contextlib.ExitStack()
        with ctx:
            cst = ctx.enter_context(tc.tile_pool(name="cst", bufs=1))
            gat_p = ctx.enter_context(tc.tile_pool(name="gat", bufs=8))
            ztmp_p = ctx.enter_context(tc.tile_pool(name="ztmp", bufs=2))
            nmt_p = ctx.enter_context(tc.tile_pool(name="nmt", bufs=3))
            nm8_p = ctx.enter_context(tc.tile_pool(name="nm8", bufs=2))
            sv_p = ctx.enter_context(tc.tile_pool(name="sv", bufs=2))
            agg_ps = ctx.enter_context(tc.tile_pool(name="aggps", bufs=1, space="PSUM"))
            z_ps = ctx.enter_context(tc.tile_pool(name="zps", bufs=2, space="PSUM"))
            sm_ps = ctx.enter_context(tc.tile_pool(name="smps", bufs=1, space="PSUM"))
            dram = ctx.enter_context(tc.tile_pool(name="dram", bufs=1, space="DRAM"))

            # ---------- resident tiles ----------
            idx_sb = cst.tile([128, NT * 8], dt.int16)
            rhs_sb = cst.tile([128, RCOLS], dt.float8e3)
            wt_sb = cst.tile([128, 18 * 128], dt.float16)
            inv_sb = cst.tile([128, NLOC], dt.float16)
            xt_all = cst.tile([128, 4 * NLOC], dt.float16)
            hT = xt_all[:, 0 * NLOC:1 * NLOC]
            hinT = xt_all[:, 1 * NLOC:2 * NLOC]
            s1T = xt_all[:, 2 * NLOC:3 * NLOC]
            s2T = xt_all[:, 3 * NLOC:4 * NLOC]
            agg_all = cst.tile([128, 3 * NLOC], dt.float16)
            aggT = [agg_all[:, a * NLOC:(a + 1) * NLOC] for a in range(3)]
            acc = cst.tile([128, NLOC], dt.float16)
            # packed bf16 smalls: ident(128) c_bf(7x128) s_bf(7) ones(1)
            sb_bf = cst.tile([128, 128 + 7 * 128 + 7 + 1], dt.float16)
            ident = sb_bf[:, 0:128]
            c_bf = [sb_bf[:, 128 + 128 * s:128 + 128 * (s + 1)] for s in range(7)]
            s_bf = [sb_bf[:, 1024 + s:1025 + s] for s in range(7)]
            ones_bf = sb_bf[:, 1031:1032]
            ones8 = cst.tile([128, 1], dt.float8e3)
            # packed f32 smalls: bn(54) arstage(258) ar_sb(258) scale(18)
            # bias(18) eps(1)
            W_F32 = 54 + 2 * ARW3 + 18 + 18 + 1
            sb_f32 = cst.tile([128, W_F32], dt.float32)
            bn_sb = sb_f32[:, 0:54]
            arstage = sb_f32[:, 54:54 + ARW3]
            ar_sb = sb_f32[:, 54 + ARW3:54 + 2 * ARW3]
            bn_scale = sb_f32[:, 54 + 2 * ARW3:54 + 2 * ARW3 + 18]
            bn_bias = sb_f32[:, 54 + 2 * ARW3 + 18:54 + 2 * ARW3 + 36]
            eps_sv = sb_f32[:, 54 + 2 * ARW3 + 36:54 + 2 * ARW3 + 37]

            ag_in0 = dram.tile([CHL, D], dt.float8e3)
            ag_in1 = dram.tile([NLOC - CHL, D], dt.float8e3)
            ar_ins = {k: dram.tile([128, AR_W[k]], dt.float32,
                                   name=f"ar_in_{k}") for k in AR_KEYS}
            cur = {}

            # ---------- prep ----------
            nc.sync.dma_start(idx_sb[:], idxs_in[:])
            nc.sync.dma_start(rhs_sb[:], rhs_in[:])
            nc.sync.dma_start(wt_sb[:], wT_in[:])
            nc.sync.dma_start(bn_sb[:], bn_in[:])
            nc.sync.dma_start(hT[:], hT_in[:])
            nc.sync.dma_start(hinT[:], hinT_in[:])
            nc.sync.dma_start(inv_sb[:], inv_in[:])
            nc.gpsimd.memset(ones_bf[:], 1.0)
            nc.gpsimd.memset(ones8[:], 1.0)
            nc.gpsimd.memset(eps_sv[:], EPS)
            from concourse.masks import make_identity
            make_identity(nc, ident[:])

            def gather_raw(out_ap, in_view, c0, ct, queue=0, prepare=False,
                           sem=None):
                """dma_gather of ct tiles (128 idx each), 128B fp8 elems at
                256B stride.  Mirrors BassGpSimd.dma_gather minus the
                %256 elem-size assert (stride stays %256 as ucode needs).
                queue selects the SWDGE queue AND the Q7 core pair that
                generates descriptors (ucode: cpu_id/2 == queue_num), so
                gathers on different queues descgen in parallel.  With
                prepare=True emits a PREPARE_ONLY prep (descgen now, DMA
                fires at the matching trigger_dma on the same queue)."""
                g = nc.gpsimd
                n_idx = ct * 128
                idxs_ap = idx_sb[:, 8 * c0:8 * (c0 + ct)]
                _in_ap = g.lower_ap_dma(in_view, for_custom_bir_dma=True)
                inst = g.add_instruction(mybir.InstDMAGatherAnt(
                    name=g.bass.get_next_instruction_name(),
                    ins=[*_in_ap, g.lower_ap(idxs_ap),
                         g.lower_val_access(g.to_reg(n_idx))],
                    outs=[g.lower_ap(out_ap)],
                    transpose=False, num_idxs=n_idx, elem_size=128,
                    stride_bytes_256=1, gen_mode=int(prepare),
                    single_packet=False,
                    queue_num=queue, sbuf_tokens_per_rank=0,
                    sbuf_free_dim_per_rank=0, sbuf_free_dim_pad_per_rank=0,
                    sbuf_byte_offset=0))
                if prepare:
                    inst.then_inc(sem, 16)
                    return g._track_prepare_only(inst, queue)
                return inst

            def moment_psums():
                mta = sm_ps.tile([128, 129], dt.float32, space="PSUM",
                                 tag="momCSa", bufs=1, name="mta")
                mtb = sm_ps.tile([128, 129], dt.float32, space="PSUM",
                                 tag="momCSb", bufs=1, name="mtb")
                return (mta, mtb)

            def moment_acc(mts, xt, t):
                # alternate between two psum accumulate chains (dependency
                # stalls on same-psum matmul chains cost ~300ns/inst on HW)
                mt = mts[t & 1]
                cps, sps = mt[:, 0:128], mt[:, 128:129]
                last = t >= NT_NODES - 2
                nc.tensor.matmul(cps, lhsT=xt, rhs=xt,
                                 start=(t < 2), stop=last,
                                 skip_group_check=True)
                nc.tensor.matmul(sps, lhsT=xt, rhs=ones8[:],
                                 start=False, stop=last,
                                 skip_group_check=True)

            def moment_evict(mts, stg, blk):
                co = 129 * blk
                mrgb = ztmp_p.tile([128, 129], dt.float32, tag="mrgb")
                nc.scalar.activation(mrgb[:], mts[1][:], AF.Copy, bias=0.0,
                                     scale=1.0)
                mrg = ztmp_p.tile([128, 129], dt.float32, tag="mrg")
                nc.vector.tensor_tensor(out=mrg[:], in0=mts[0][:],
                                        in1=mrgb[:], op=OP.add)
                nc.vector.tensor_scalar_mul(stg[:, co:co + 129], mrg[:],
                                            1.0 / N)

            def moments_from_cm(xT, stg, blk, sink=None):
                """Moments of a ch-major [128, NLOC] state via PE transposes.
                sink(tn, nm_ap, wv) optionally consumes node-major tiles."""
                mts = moment_psums()
                pend = []

                def _consume(tn, tp, wv):
                    nm8 = nm8_p.tile([128, 128], dt.float8e3, tag="nm8")
                    if wv < 128:
                        nc.vector.memset(nm8[96:, :], 0)
                    nc.scalar.activation(nm8[:wv, :], tp[:wv, :], AF.Copy,
                                         bias=0.0, scale=1.0)
                    moment_acc(mts, nm8[:], tn)
                    if sink is not None:
                        sink(tn, nm8, wv)

                for tn in range(NT_NODES):
                    wv = min(128, NLOC - 128 * tn)
                    tp = sm_ps.tile([128, 128], dt.float16, space="PSUM", tag="tp",
                                    bufs=2)
                    nc.tensor.transpose(tp[:wv, :], xT[:, 128 * tn:128 * tn + wv],
                                        ident[:])
                    pend.append((tn, tp, wv))
                    if len(pend) > 1:
                        _consume(*pend.pop(0))
                _consume(*pend.pop(0))
                moment_evict(mts, stg, blk)

            # ---------- gather prefetch machinery ----------
            # Shared per-pass call plan: same structure every pass, greedy
            # per-queue tile balance.  Preps (descgen on the queue's Q7
            # pair) are emitted ahead of time via pump(); the agg phase
            # fires them with trigger_dma in the same per-queue order.
            call_plan = []        # (c, qpar, b, t0g0, ct, qsel)
            _gq = [0, 0, 0, 0]
            for _c in range(2):
                for _b in range(NBLK):
                    for _qp in range(2):
                        _t0, _ntg = int(tbase[_b, _qp, _c]), int(ntq[_b, _qp, _c])
                        if _ntg == 0:
                            continue
                        _ncall = (_ntg + GCH - 1) // GCH
                        _g0 = 0
                        for _i in range(_ncall):
                            _ct = _ntg // _ncall + (1 if _i < _ntg % _ncall
                                                    else 0)
                            _qs = min(range(4), key=lambda j: _gq[j])
                            _gq[_qs] += _ct
                            call_plan.append((_c, _qp, _b, _t0 + _g0, _ct, _qs))
                            _g0 += _ct
            def queue_phase(tbl0, tbl1):
                """Record the table views for the next agg pass."""
                views = [(tbl0[:, 0:128], tbl0[:, 128:256]),
                         (tbl1[:, 0:128], tbl1[:, 128:256])]
                return views

            # ---------- aggregation ----------
            def agg_phase(a, views, sidework=None):
                """agg into aggT[a]: per-call dma_gather (round-robin SWDGE
                queues, descgen parallel on 4 Q7 pairs) + one-hot matmuls
                accumulating in PSUM.  Pass 0 evicts invdeg-scaled partials
                into aggT (fp16); pass 1 adds on top."""
                dst = aggT[a]
                mts = moment_psums() if stage != 2 else None
                mom_state = [0]

                def mom_flush(upto):
                    while mom_state[0] * 128 < upto:
                        tn = mom_state[0]
                        wv = min(128, NLOC - 128 * tn)
                        tp = sm_ps.tile([128, 128], dt.float16, space="PSUM",
                                        tag="tp", bufs=2)
                        nc.tensor.transpose(tp[:wv, :],
                                            dst[:, 128 * tn:128 * tn + wv],
                                            ident[:])
                        nm8 = nm8_p.tile([128, 128], dt.float8e3, tag="nm8")
                        if wv < 128:
                            nc.vector.memset(nm8[96:, :], 0)
                        nc.scalar.activation(nm8[:wv, :], tp[:wv, :], AF.Copy,
                                             bias=0.0, scale=1.0)
                        moment_acc(mts, nm8[:], tn)
                        mom_state[0] += 1

                ci = 0
                for c in range(2):
                    for b in range(NBLK):
                        Pa = agg_ps.tile([128, 512], dt.float32, space="PSUM",
                                         tag="Pa", name="Pa")
                        Pb = agg_ps.tile([128, 512], dt.float32, space="PSUM",
                                         tag="Pb", name="Pb")
                        Ps = (Pa, Pb)
                        nmm = int(ntq[b, 0, c] + ntq[b, 1, c])
                        wsn = min((b + 1) * WPB, NWIN) - b * WPB
                        for Px in (Pa, Pb):
                            nc.tensor.matmul(
                                Px[:, 0:WIN * wsn], lhsT=rhs_sb[:, 0:128],
                                rhs=rhs_sb[:, ZOFF:ZOFF + WIN * wsn],
                                start=True, stop=False, skip_group_check=True)
                        done = 0
                        while ci < len(call_plan) and call_plan[ci][0] == c \
                                and call_plan[ci][2] == b:
                            _, qp, _, t0, ct, qs = call_plan[ci]
                            g = gat_p.tile([128, ct, 128], dt.float8e3,
                                           tag="gat")
                            gather_raw(g[:], views[c][qp], t0, ct, queue=qs)
                            for i in range(ct):
                                t = t0 + i
                                w0, k, rco = tinfo[t]
                                co = WIN * (w0 - b * WPB)
                                nc.tensor.matmul(
                                    Ps[done & 1][:, co:co + WIN * k],
                                    lhsT=g[:, i, :],
                                    rhs=rhs_sb[:, rco:rco + WIN * k],
                                    start=False,
                                    stop=(done >= nmm - 2),
                                    skip_group_check=True)
                                done += 1
                            ci += 1
                        nbc = min(512, NLOC - 512 * b)
                        dchunk = dst[:, 512 * b:512 * b + nbc]
                        cpb = ztmp_p.tile([128, 512], dt.float16, tag="psum2",
                                          name="cpb")
                        nc.scalar.activation(cpb[:, :nbc], Pb[:, :nbc], AF.Copy,
                                             bias=0.0, scale=1.0)
                        ps_sum = ztmp_p.tile([128, 512], dt.float16, tag="psum3",
                                             name="ps_sum")
                        nc.vector.tensor_tensor(
                            out=ps_sum[:, :nbc], in0=Pa[:, :nbc],
                            in1=cpb[:, :nbc], op=OP.add)
                        if c == 0:
                            nc.vector.tensor_tensor(
                                out=dchunk, in0=ps_sum[:, :nbc],
                                in1=inv_sb[:, 512 * b:512 * b + nbc], op=OP.mult)
                            continue
                        pt = ztmp_p.tile([128, 512], dt.float16, tag="zt")
                        nc.vector.tensor_tensor(
                            out=pt[:, :nbc], in0=ps_sum[:, :nbc],
                            in1=inv_sb[:, 512 * b:512 * b + nbc], op=OP.mult)
                        nc.vector.tensor_tensor(
                            out=dchunk, in0=dchunk, in1=pt[:, :nbc], op=OP.add)
                        if stage != 2 and b > 0:
                            mom_flush(512 * b)     # lag one block

                if stage != 2:
                    mom_flush(NLOC)
                return mts

            # ---------- collectives ----------
            def fire_ar(ar_in, ar_out, stg_ap):
                nc.sync.dma_start(ar_in[:], stg_ap)
                nc.gpsimd.collective_compute(
                    "AllReduce", OP.add, replica_groups=[list(range(NC))],
                    ins=[ar_in.opt()], outs=[ar_out.opt()])

            def unpack_ar(ar_out, blocks):
                """blocks: list of (block idx in ar_out, state id)"""
                w = 129 * len(blocks)
                nc.sync.dma_start(ar_sb[:, :w], ar_out[:])
                for i, (blk, sid) in enumerate(blocks):
                    co = 129 * i
                    nc.vector.tensor_copy(c_bf[sid][:], ar_sb[:, co:co + 128])
                    nc.vector.tensor_copy(s_bf[sid][:], ar_sb[:, co + 128:co + 129])

            # ---------- BN params for one (m, c) ----------
            def bn_params(mc, sid):
                wt = wt_sb[:, 128 * mc:128 * (mc + 1)]
                wgam = bn_sb[:, 3 * mc + 0:3 * mc + 1]
                wbeta = bn_sb[:, 3 * mc + 1:3 * mc + 2]
                blin = bn_sb[:, 3 * mc + 2:3 * mc + 3]
                vps = sm_ps.tile([128, 128], dt.float32, space="PSUM",
                                 tag="tp", bufs=2)
                nc.tensor.matmul(vps[:], lhsT=c_bf[sid][:], rhs=wt, start=True,
                                 stop=True)
                vsb = nmt_p.tile([128, 128], dt.float16, tag="nmt")
                nc.vector.tensor_copy(vsb[:], vps[:])
                msb = nmt_p.tile([128, 128], dt.float16, tag="nmt")
                nc.vector.tensor_tensor(out=msb[:], in0=vsb[:], in1=wt, op=OP.mult)
                bnv = sm_ps.tile([128, 2], dt.float32, space="PSUM",
                                 tag="tp", bufs=2, name="bnv")
                d2, ws_ = bnv[:, 0:1], bnv[:, 1:2]
                nc.tensor.matmul(d2, lhsT=msb[:], rhs=ones_bf[:], start=True,
                                 stop=True, skip_group_check=True)
                nc.tensor.matmul(ws_, lhsT=wt, rhs=s_bf[sid][:], start=False,
                                 stop=True, skip_group_check=True)
                sv = sv_p.tile([128, 10], dt.float32, tag="sv")
                mu, t1, t2, e2, mu2, var, sd, rv, t4, t5 = (
                    sv[:, i:i + 1] for i in range(10))
                nc.vector.tensor_tensor(out=mu, in0=ws_[:], in1=blin, op=OP.add)
                nc.vector.tensor_scalar(out=t1, in0=ws_[:], scalar1=2.0,
                                        scalar2=blin, op0=OP.mult, op1=OP.add)
                nc.vector.tensor_tensor(out=t2, in0=t1, in1=blin, op=OP.mult)
                nc.vector.tensor_tensor(out=e2, in0=d2[:], in1=t2, op=OP.add)
                nc.vector.tensor_tensor(out=mu2, in0=mu, in1=mu, op=OP.mult)
                nc.vector.tensor_tensor(out=var, in0=e2, in1=mu2, op=OP.subtract)
                nc.scalar.activation(sd, var, AF.Sqrt, bias=eps_sv, scale=1.0)
                nc.vector.reciprocal(rv, sd)
                nc.vector.tensor_tensor(out=bn_scale[:, mc:mc + 1], in0=wgam,
                                        in1=rv, op=OP.mult)
                nc.vector.tensor_tensor(out=t4, in0=blin, in1=mu, op=OP.subtract)
                nc.vector.tensor_tensor(out=t5, in0=bn_scale[:, mc:mc + 1],
                                        in1=t4, op=OP.mult)
                nc.vector.tensor_tensor(out=bn_bias[:, mc:mc + 1], in0=wbeta,
                                        in1=t5, op=OP.add)

            # ---------- z partials ----------
            zstate = {"first": [True] * len(ZC)}

            def z_chunk(r, m, c, ci):
                j = m - ROUND_MS[r][0]
                mc = m * 3 + c
                rx = (aggT[j] if c == 0 else
                      [hT, s1T, s2T][j] if c == 1 else hinT)
                lw = wt_sb[:, 128 * mc:128 * (mc + 1)]
                off, cw = 512 * ci, ZC[ci]
                zp = z_ps.tile([128, 512], dt.float32, space="PSUM",
                               tag="zp")
                nc.tensor.matmul(zp[:, :cw], lhsT=lw,
                                 rhs=rx[:, off:off + cw],
                                 start=True, stop=True)
                if zstate["first"][ci]:
                    nc.scalar.activation(
                        acc[:, off:off + cw], zp[:, :cw], AF.Relu,
                        bias=bn_bias[:, mc:mc + 1],
                        scale=bn_scale[:, mc:mc + 1])
                    zstate["first"][ci] = False
                else:
                    zt = ztmp_p.tile([128, 512], dt.float16, tag="zt")
                    nc.scalar.activation(
                        zt[:, :cw], zp[:, :cw], AF.Relu,
                        bias=bn_bias[:, mc:mc + 1],
                        scale=bn_scale[:, mc:mc + 1])
                    nc.vector.tensor_tensor(
                        out=acc[:, off:off + cw],
                        in0=acc[:, off:off + cw],
                        in1=zt[:, :cw], op=OP.add)

            def z_partial(r, pairs):
                for m, c in pairs:
                    for ci in range(len(ZC)):
                        z_chunk(r, m, c, ci)

            # ---------- fused round tail: z c0 + out + incremental
            # finalize (sT copy, moments, fp8 cast, chunked AllGather) ----
            def finish_round(r):
                """c0 z-matmuls chunk by chunk; finalize work (sT copy,
                transpose, moments, fp8 rows) LAGS one chunk so engines
                pipeline instead of ping-ponging; AG0 fires once the first
                CHL rows are on DRAM (r<2)."""
                ms = ROUND_MS[r]
                sT = [s1T, s2T][r] if r < 2 else None
                mts = moment_psums() if r < 2 else None
                fin_state = [0]

                def fin_flush(upto):
                    while fin_state[0] * 128 < upto:
                        tn = fin_state[0]
                        wv = min(128, NLOC - 128 * tn)
                        tp = sm_ps.tile([128, 128], dt.float16,
                                        space="PSUM", tag="tp", bufs=2)
                        nc.tensor.transpose(
                            tp[:wv, :], sT[:, 128 * tn:128 * tn + wv],
                            ident[:])
                        nm8 = nm8_p.tile([128, 128], dt.float8e3, tag="nm8")
                        if wv < 128:
                            nc.vector.memset(nm8[96:, :], 0)
                        nc.vector.tensor_copy(nm8[:wv, :], tp[:wv, :])
                        moment_acc(mts, nm8[:], tn)
                        n0 = 128 * tn
                        if n0 + wv <= CHL:
                            nc.sync.dma_start(ag_in0[n0:n0 + wv, :],
                                              nm8[:wv, :])
                        else:
                            nc.sync.dma_start(
                                ag_in1[n0 - CHL:n0 - CHL + wv, :],
                                nm8[:wv, :])
                        fin_state[0] += 1
                        if fin_state[0] * 128 == CHL:
                            t0, _ = cur["tbls"][r]
                            nc.gpsimd.collective_compute(
                                "AllGather", OP.bypass,
                                replica_groups=[list(range(NC))],
                                ins=[ag_in0.opt()], outs=[t0.opt()])

                off = 0
                for ci, cw in enumerate(ZC):
                    for j, m in enumerate(ms):
                        mc = m * 3 + 0
                        lw = wt_sb[:, 128 * mc:128 * (mc + 1)]
                        zp = z_ps.tile([128, 512], dt.float32, space="PSUM",
                                       tag="zp")
                        nc.tensor.matmul(zp[:, :cw], lhsT=lw,
                                         rhs=aggT[j][:, off:off + cw],
                                         start=True, stop=True)
                        zt = ztmp_p.tile([128, 512], dt.float16, tag="zt")
                        nc.scalar.activation(zt[:, :cw], zp[:, :cw], AF.Relu,
                                             bias=bn_bias[:, mc:mc + 1],
                                             scale=bn_scale[:, mc:mc + 1])
                        nc.vector.tensor_tensor(
                            out=acc[:, off:off + cw], in0=acc[:, off:off + cw],
                            in1=zt[:, :cw], op=OP.add)
                    nc.sync.dma_start(out_cm[r][:, off:off + cw],
                                      acc[:, off:off + cw])
                    if r < 2:
                        nc.vector.tensor_copy(sT[:, off:off + cw],
                                              acc[:, off:off + cw])
                        if ci > 0:
                            fin_flush(off)        # lag one chunk
                    off += cw
                if r < 2:
                    fin_flush(NLOC)
                    _, t1_ = cur["tbls"][r]
                    nc.gpsimd.collective_compute(
                        "AllGather", OP.bypass, replica_groups=[list(range(NC))],
                        ins=[ag_in1.opt()], outs=[t1_.opt()])
                    moment_evict(mts, arstage, 0)
                    key = f"s{r + 1}"
                    fire_ar(ar_ins[key], cur["ar"][key], arstage[:, 0:129])
                    return cur["tbls"][r]

            def agg_and_bn(r, views, sidework=None):
                """agg phase r (with interleaved sidework on the otherwise
                descgen-bound stretch), then the agg-moment AR; leftover
                sidework flushes after the AR fires (its latency shadow)."""
                mts = agg_phase(r, views, sidework)
                blk = 2 if r == 0 else 1
                key = f"a{r}"
                if stage != 2:
                    moment_evict(mts, arstage, blk)
                if stage not in (0, 2):
                    fire_ar(ar_ins[key], cur["ar"][key],
                            arstage[:, 129 * blk:129 * (blk + 1)])
                while sidework:
                    sidework.pop(0)()

            # ================= main schedule =================
            def emit(it, last):
                zstate["first"] = [True] * len(ZC)
                s0 = queue_phase(th0_in[:, :], th1_in[:, :])
                moments_from_cm(hT, arstage, 0)
                moments_from_cm(hinT, arstage, 1)
                if stage not in (0, 2):
                    fire_ar(ar_ins["hh"], cur["ar"]["hh"], arstage[:, 0:258])

                # round 0: identity/skip z-branches run as sidework inside
                # the (descgen-bound) agg phase, once the early h/hin
                # moment-AR lands.
                s1 = queue_phase(*cur["tbls"][0])
                sw = []
                if stage not in (0, 1, 2):
                    def _unp0():
                        unpack_ar(cur["ar"]["hh"], [(0, S_H), (1, S_HIN)])
                        bn_params(1, S_H)
                        bn_params(2, S_HIN)
                    sw.append(lambda: None)   # let AR-hh land first
                    sw.append(_unp0)
                    for ci in range(len(ZC)):
                        sw.append(lambda ci=ci: (z_chunk(0, 0, 1, ci),
                                                 z_chunk(0, 0, 2, ci)))
                agg_and_bn(0, s0, sw)
                if stage in (0, 1, 2):
                    off = 0
                    for cw in ZC:
                        nc.vector.tensor_copy(acc[:, off:off + cw],
                                              aggT[0][:, off:off + cw])
                        off += cw
                    nc.sync.dma_start(out_cm[0], acc[:])
                    return
                unpack_ar(cur["ar"]["a0"], [(0, S_A0)])
                bn_params(0, S_A0)
                tbl10, tbl11 = finish_round(0)
                if stage == 3:
                    return
                zstate["first"] = [True] * len(ZC)

                # round 1: identity branch on h under AG+gather; the rest as
                # agg sidework (s1-stat unpack a few blocks in, once AR-s1
                # lands).
                bn_params(1 * 3 + 1, S_H)
                bn_params(1 * 3 + 2, S_HIN)
                bn_params(2 * 3 + 2, S_HIN)
                z_partial(1, [(1, 1)])
                s2 = queue_phase(*cur["tbls"][1])
                sw = [lambda ci=ci: z_chunk(1, 1, 2, ci)
                      for ci in range(len(ZC))]

                def _unp1():
                    unpack_ar(cur["ar"]["s1"], [(0, S_S1)])
                    bn_params(2 * 3 + 1, S_S1)
                sw.insert(10, _unp1)
                sw += [lambda ci=ci: z_chunk(1, 2, 2, ci)
                       for ci in range(len(ZC))]
                sw += [lambda ci=ci: z_chunk(1, 2, 1, ci)
                       for ci in range(len(ZC))]
                agg_and_bn(1, s1, sw)
                unpack_ar(cur["ar"]["a1"], [(0, S_A1)])
                bn_params(1 * 3 + 0, S_A0)
                bn_params(2 * 3 + 0, S_A1)
                tbl20, tbl21 = finish_round(1)
                zstate["first"] = [True] * len(ZC)

                # round 2
                bn_params(3 * 3 + 1, S_H)
                bn_params(3 * 3 + 2, S_HIN)
                bn_params(4 * 3 + 1, S_S1)
                bn_params(4 * 3 + 2, S_HIN)
                bn_params(5 * 3 + 2, S_HIN)
                z_partial(2, [(3, 1), (4, 1)])
                sw = [lambda ci=ci: z_chunk(2, 3, 2, ci)
                      for ci in range(len(ZC))]

                def _unp2():
                    unpack_ar(cur["ar"]["s2"], [(0, S_S2)])
                    bn_params(5 * 3 + 1, S_S2)
                sw.insert(10, _unp2)
                sw += [lambda ci=ci: z_chunk(2, 4, 2, ci)
                       for ci in range(len(ZC))]
                sw += [lambda ci=ci: z_chunk(2, 5, 2, ci)
                       for ci in range(len(ZC))]
                sw += [lambda ci=ci: z_chunk(2, 5, 1, ci)
                       for ci in range(len(ZC))]
                agg_and_bn(2, s2, sw)
                unpack_ar(cur["ar"]["a2"], [(0, S_A2)])
                bn_params(3 * 3 + 0, S_A0)
                bn_params(4 * 3 + 0, S_A1)
                bn_params(5 * 3 + 0, S_A2)
                finish_round(2)

            for it in range(iters):
                cur["tbls"] = [
                    (dram.tile([CH0 // 2, 256], dt.float8e3, addr_space="Shared",
                               name=f"tbl{r}0_{it}"),
                     dram.tile([CH1 // 2, 256], dt.float8e3, addr_space="Shared",
                               name=f"tbl{r}1_{it}"))
                    for r in range(2)]
                cur["ar"] = {k: dram.tile([128, AR_W[k]], dt.float32,
                                          addr_space="Shared",
                                          name=f"ar_{k}_{it}")
                             for k in AR_KEYS}
                emit(it, it == iters - 1)

    nc.compile()
    return nc


# ---------------------------------------------------------------- entry point
def prepare(edge_index, h, h_in, weights, W, b, gamma, beta, stage=99, iters=1):
    struct = preprocess(edge_index)
    th0, th1, wT, bn, per_core_s = make_host_inputs(
        h, h_in, weights, W, b, gamma, beta)
    in_maps = []
    for k in range(NC):
        idxw, rhs, inv_bc = struct["per_core"][k]
        in_maps.append(dict(
            table_h0=th0, table_h1=th1, idxs=idxw, rhs=rhs, wT=wT,
            bn_small=bn, inv_bc=inv_bc,
            hT=per_core_s[k]["hT"], hinT=per_core_s[k]["hinT"]))
    nc = build(struct, stage=stage, iters=iters)
    return nc, in_maps


def assemble(results):
    out = np.empty((3, N, D), np.float32)
    for k in range(NC):
        cm = results[k]["out_cm"]
        for r in range(3):
            out[r, k * NLOC:(k + 1) * NLOC, :] = cm[r].T
    return out


def kernel(edge_index, h, h_in, weights, W, b, gamma, beta):
    from concourse.bass_utils import run_bass_kernel_spmd
    nc, in_maps = prepare(np.asarray(edge_index), h, h_in,
                          np.asarray(weights, np.float32),
                          np.asarray(W, np.float32), np.asarray(b, np.float32),
                          np.asarray(gamma, np.float32),
                          np.asarray(beta, np.float32))
    res = run_bass_kernel_spmd(nc, in_maps, core_ids=list(range(NC)))
    return assemble(res.results)

